# revision 19
# baseline (speedup 1.0000x reference)
"""Trainium2 Bass kernel: DepthSeparableConv2d (dw3x3 + BN + ReLU + map-cut,
pw 1x1 + BN + ReLU + map-cut), data-parallel over batch on 8 NeuronCores.

Host side folds all the small weight algebra (BN scales into conv weights,
pw transpose, biases, the dw-cut threshold) in numpy and zero-pads x to
58x58 per channel; the device kernel is a pure streaming pipeline per core
(4 images):

  - depthwise 3x3 conv in SIX TensorEngine passes per 448-px tile instead
    of nine: the three horizontal tap pairs (di,-1)+(di,+1) are fused into
    one pass each using 128x64 COLUMN-TILED matmuls.  Two concurrent
    col-tile matmuls (tile_position (0,0) and (0,64)) contract 128-deep
    streams XA = [x_lo; x_lo shifted +2 elems] and XB = [x_hi; x_hi +2]
    (partition-duplicated planes built by the input DMA from the padded
    HBM tensor), so one pass applies BOTH taps of a pair for all 128
    channels.  The three center taps ride the same mode with zeroed
    hi-row weights.  Measured: the second col-tile matmul executes in the
    ~4ns shadow of the first, so a 2-tap pass costs one matmul slot.
  - zero padding lives in the HBM layout (host-padded), so every matmul
    is a full 8x56 window -- no border clipping, no edge fixups.
  - PSUM tiles are 2-bank [128,1024] pairs; each pair drains in ONE
    Scalar/Vector instruction (relu(psum+bias), bf16), and the dw map-cut
    stat is one VectorE XY-reduce straight from PSUM for the last image.
    keep is folded into the pointwise lhsT halves.
  - pointwise 1x1: 2 chunks of 128 out-channels, each emitted as two
    concurrent 128x64 col-tile matmuls so the PE never switches tiling
    mode (mode switches drain the array); paired drains
    relu(psum+bias2) -> bf16 z, ONE output DMA per chunk; the last
    image's chunks go out as two halves on two queues.
  - pw map-cut is applied ON HOST (exact max >= 0.001 test in numpy);
    output DMA is bf16 (half traffic), host casts to fp32.
  - schedule: pw(n) is emitted BETWEEN pairs 2 and 3 of dw(n+1); image 0's
    input load is split into 3 row-chunks x 2 planes across the sync /
    gpsimd / scalar / vector queues so the first dw pair starts as early
    as possible.
"""

import numpy as np

B, C_IN, C_OUT, H, W = 32, 128, 256, 56, 56
N_CORES = 8
BPC = B // N_CORES          # images per core
HW = H * W                  # 3136
TILE_ROWS = 8               # output rows per 448-px sub-tile
NT = H // TILE_ROWS         # 7 sub-tiles per image
TN = TILE_ROWS * W          # 448 pixels per sub-tile
BN_EPS = 1e-5
DW_THRESH = 4.0
PW_THRESH = 0.001

HP, WP = H + 2, W + 2       # padded image 58x58
PHW = HP * WP               # 3364
PXW = PHW + 2               # host row length (2 zeros so the +2-shifted
                            # plane load stays in bounds)

# pairs of 448-px sub-tiles sharing one 2-bank PSUM tile
PAIRS = [(0, 1), (2, 3), (4, 5), (6,)]

# dw passes: (kind, di) with kind 'c' = center tap (0 or +-1, 0) or
# 'p' = horizontal pair (di,-1)+(di,+1).  Center (0,0) first: its two
# col-tile matmuls carry start=True and cover the full tile region.
DW_PASSES = [("c", 0), ("c", -1), ("c", 1), ("p", -1), ("p", 0), ("p", 1)]

_CACHE = {}


def _build():
    import concourse.bacc as bacc
    import concourse.tile as tile
    import concourse.mybir as mybir

    f32 = mybir.dt.float32
    bf16 = mybir.dt.bfloat16
    Alu = mybir.AluOpType
    Act = mybir.ActivationFunctionType

    nc = bacc.Bacc("TRN2", target_bir_lowering=False, debug=False,
                   enable_asserts=False, num_devices=N_CORES)

    # input planes prebuilt on host in exact SBUF layout: plane 0 =
    # [x_lo; x_lo shifted +2 elems], plane 1 = [x_hi; x_hi +2] -- every
    # device read is a full-width, aligned, contiguous DMA
    xp_d = nc.dram_tensor("xq", [BPC, 2, C_IN, PHW], bf16,
                          kind="ExternalInput").ap()
    wc_d = nc.dram_tensor("wcol", [C_IN, 6, 2, 64], bf16, kind="ExternalInput").ap()
    # bias1 / thr1 / bias2-lo / bias2-hi packed as one [C_IN, 4] tensor
    bv_d = nc.dram_tensor("biasv", [C_IN, 4], f32, kind="ExternalInput").ap()
    lw_d = nc.dram_tensor("lhsTb", [C_IN, C_OUT], bf16, kind="ExternalInput").ap()
    z_d = nc.dram_tensor("z", [BPC, C_OUT, H, W], bf16, kind="ExternalOutput").ap()

    TPOS = [(0, 0), (0, 64)]
    HALVES = [(0, 64), (64, 128)]

    with tile.TileContext(nc) as tc:
        with tc.tile_pool(name="const", bufs=1) as cp, \
             tc.tile_pool(name="xb", bufs=3) as xbp, \
             tc.tile_pool(name="y", bufs=3) as yp, \
             tc.tile_pool(name="z", bufs=4) as zp, \
             tc.tile_pool(name="small", bufs=8) as sp, \
             tc.tile_pool(name="dwps", bufs=2, space="PSUM") as dwps_pool, \
             tc.tile_pool(name="pwps", bufs=2, space="PSUM") as pwps_pool:

            # ---- startup: weights on gpsimd queue; image 0 in 3 row-chunks
            # x 2 planes spread across 4 queues so pair 0 can start early ----
            wcol = cp.tile([128, 6, 2, 64], bf16)
            nc.gpsimd.dma_start(wcol[:], wc_d)
            bv = cp.tile([128, 4], f32)
            nc.gpsimd.dma_start(bv[:], bv_d)
            bias1 = bv[:, 0:1]
            thr1 = bv[:, 1:2]
            bias2 = [bv[:, 2:3], bv[:, 3:4]]
            lhsT_base = cp.tile([128, C_OUT], bf16)
            nc.gpsimd.dma_start(lhsT_base[:], lw_d)

            xb0 = xbp.tile([128, 2, PHW], bf16, name="xbt")

            def load_img(qs, n, xab, chunks):
                """DMA image n's two prebuilt planes; one full-partition
                aligned DMA per (chunk, plane)."""
                k = 0
                for (a, b) in chunks:
                    for pl in range(2):
                        qs[k % len(qs)].dma_start(
                            xab[:, pl, a:b], xp_d[n, pl, :, a:b])
                        k += 1

            # chunk 1 covers pair 0 (rows 0-17), chunk 2 pair 1 (rows to 33):
            # both split across sync+scalar, which start their DGEs first.
            # chunk 3 (rows 34-57, needed ~6us later) rides gpsimd behind the
            # small weight loads.
            load_img([nc.sync, nc.scalar], 0, xb0, [(0, 18 * WP)])
            load_img([nc.sync, nc.scalar], 0, xb0, [(18 * WP, 34 * WP)])
            load_img([nc.gpsimd], 0, xb0, [(34 * WP, PHW)])

            # warm the PE HAM clock while the first DMAs are in flight
            # (col-tiled like everything else: mode switches drain the PE)
            warm = cp.tile([128, 448], bf16)
            nc.vector.memset(warm[:], 0.0)
            wps = pwps_pool.tile([128, 1024], f32, name="pwps")

            def warm_mm(n_reps, psum):
                for _ in range(n_reps):
                    for ti, (p0, p1) in enumerate(HALVES):
                        nc.tensor.matmul(psum[p0:p1, 0:448], warm[:, 0:64],
                                         warm[:], start=True, stop=True,
                                         tile_position=TPOS[ti],
                                         skip_group_check=True)

            warm_mm(7, wps)

            state = {"dr": 0}

            def drain_op(dst, src, bias, rot):
                state["dr"] += 1
                eng = rot[state["dr"] % len(rot)]
                if eng is nc.scalar:
                    nc.scalar.activation(dst, src, Act.Relu,
                                         bias=bias, scale=1.0)
                else:
                    eng.tensor_scalar(dst, src, bias, 0.0,
                                      Alu.add, Alu.max)

            def emit_dw_pair(img, pi, stat_from_psum=False):
                """one PSUM pair of the depthwise conv: 6 col-tiled passes
                per 448-px tile, keep-stat reduce, paired drain."""
                xab, yb, partdw = img["xb"], img["yb"], img["partdw"]
                pair = img["pairs"][pi]
                ps = dwps_pool.tile([128, 1024], f32, name="dwps")
                x3 = [xab[:, 0, :].rearrange("c (h w) -> c h w", h=HP),
                      xab[:, 1, :].rearrange("c (h w) -> c h w", h=HP)]
                for k, tt in enumerate(pair):
                    r0 = tt * TILE_ROWS
                    ps3 = ps[:, k * 512:k * 512 + TN].rearrange(
                        "c (h w) -> c h w", h=TILE_ROWS)
                    for pidx, (kind, di) in enumerate(DW_PASSES):
                        rlo = r0 + 1 + di
                        cl = 1 if kind == "c" else 0
                        first = (pidx == 0)
                        last = (pidx == len(DW_PASSES) - 1)
                        for ti, (p0, p1) in enumerate(HALVES):
                            nc.tensor.matmul(
                                ps3[p0:p1],
                                wcol[:, pidx, ti, :],
                                x3[ti][:, rlo:rlo + TILE_ROWS, cl:cl + W],
                                start=first, stop=last,
                                tile_position=TPOS[ti],
                                skip_group_check=True)
                npair = len(pair)
                src = ps[:, 0:npair * 512].rearrange(
                    "c (b x) -> c b x", b=npair)[:, :, 0:TN]
                c0 = pair[0] * TN
                dst = yb[:, c0:c0 + npair * TN].rearrange(
                    "c (b x) -> c b x", b=npair)
                if stat_from_psum:
                    nc.vector.tensor_reduce(partdw[:, pi:pi + 1], src,
                                            axis=mybir.AxisListType.XY,
                                            op=Alu.max)
                drain_op(dst, src, bias1, (nc.scalar, nc.scalar, nc.vector))
                if not stat_from_psum:
                    nc.vector.tensor_reduce(partdw[:, pi:pi + 1],
                                            yb[:, c0:c0 + npair * TN],
                                            axis=mybir.AxisListType.X,
                                            op=Alu.max)
                if pi == 2:
                    nc.vector.tensor_reduce(img["mxa"][:], partdw[:, 0:3],
                                            axis=mybir.AxisListType.X,
                                            op=Alu.max)

            def emit_chain(img, thr=float(DW_THRESH)):
                """keep1 -> masked lhsT halves (all on VectorE)."""
                mx1 = sp.tile([128, 1], f32, name="mx1")
                nc.vector.tensor_max(mx1[:], img["mxa"][:],
                                     img["partdw"][:, 3:4])
                keep1 = sp.tile([128, 1], f32, name="keep1")
                nc.vector.tensor_scalar(keep1[:], mx1[:], thr,
                                        None, Alu.is_ge)
                for m in range(2):
                    lm = sp.tile([128, 128], bf16, name=f"lhsTm{m}")
                    nc.vector.tensor_scalar(
                        lm[:], lhsT_base[:, m * 128:(m + 1) * 128], keep1[:],
                        None, Alu.mult)
                    img["lhsTm"].append(lm)

            def emit_pw(img, pools, tail=False, chunks=(0, 1)):
                """pw out-channel chunks, col-tiled matmuls + paired
                drains.  Output DMA per chunk; the last image (tail=True)
                streams each drained pair-group out immediately."""
                n, yb = img["n"], img["yb"]
                for m in chunks:
                    zrow = z_d[n, m * 128:(m + 1) * 128].rearrange(
                        "c h w -> c (h w)")
                    lhsTm = img["lhsTm"][m]
                    zt = zp.tile([128, HW], bf16, name="zt")
                    for pj, pair in enumerate(PAIRS):
                        pool = pools[pj % len(pools)]
                        ps = pool.tile([128, 1024], f32,
                                       name="dwps" if pool is dwps_pool
                                       else "pwps")
                        for k, tt in enumerate(pair):
                            for ti, (p0, p1) in enumerate(HALVES):
                                nc.tensor.matmul(
                                    ps[p0:p1, k * 512:k * 512 + TN],
                                    lhsTm[:, p0:p1],
                                    yb[:, tt * TN:(tt + 1) * TN],
                                    start=True, stop=True,
                                    tile_position=TPOS[ti],
                                    skip_group_check=True)
                        npair = len(pair)
                        c0 = pair[0] * TN
                        src = ps[:, 0:npair * 512].rearrange(
                            "c (b x) -> c b x", b=npair)[:, :, 0:TN]
                        dst = zt[:, c0:c0 + npair * TN].rearrange(
                            "c (b x) -> c b x", b=npair)
                        drain_op(dst, src, bias2[m],
                                 (nc.scalar, nc.scalar, nc.scalar, nc.vector))
                        if tail:
                            # stream each drained pair-group out immediately,
                            # alternating queues, so the final HBM burst is
                            # four small pieces instead of two big ones
                            qa, qb = ((nc.sync, nc.scalar) if m == 0
                                      else (nc.scalar, nc.sync))
                            q = qa if pj % 2 == 0 else qb
                            q.dma_start(zrow[:, c0:c0 + npair * TN],
                                        zt[:, c0:c0 + npair * TN])
                    if not tail:
                        # chunk 0 on sync, chunk 1 on scalar: keeps the sync
                        # queue's per-image byte load under the image period
                        (nc.sync if m == 0 else nc.scalar).dma_start(
                            zrow[:], zt[:])

            def new_img(n, xab):
                if n == 0:
                    pairs = [PAIRS[0], PAIRS[1], PAIRS[3], PAIRS[2]]
                elif n == BPC - 1:
                    pairs = PAIRS
                else:
                    pairs = [PAIRS[3]] + PAIRS[0:3]
                return {"n": n, "xb": xab, "pairs": pairs,
                        "yb": yp.tile([128, HW], bf16, name="ybt"),
                        "partdw": sp.tile([128, 4], f32, name="partdw"),
                        "mxa": sp.tile([128, 1], f32, name="mxa"),
                        "lhsTm": []}

            imgs = [None] * BPC
            imgs[0] = new_img(0, xb0)
            # image 1's planes load right behind image 0's chunks (sync +
            # scalar queues are FIFO, so these start as soon as img0 is in)
            xab1 = xbp.tile([128, 2, PHW], bf16, name="xbt")
            load_img([nc.sync, nc.scalar], 1, xab1, [(0, PHW)])
            imgs[1] = new_img(1, xab1)
            for n in range(BPC):
                img = imgs[n]
                emit_dw_pair(img, 0, stat_from_psum=(n == 3))
                # prefetch image n+2 == 2 on the idle gpsimd queue (its tile
                # buffer is fresh, so the DMA carries no semaphore waits --
                # waiting DMAs on the gpsimd queue hang); image 3 reuses a
                # buffer and goes on sync+scalar
                if n == 0:
                    xab = xbp.tile([128, 2, PHW], bf16, name="xbt")
                    load_img([nc.gpsimd], 2, xab, [(0, PHW)])
                    imgs[2] = new_img(2, xab)
                elif n == 1:
                    xab = xbp.tile([128, 2, PHW], bf16, name="xbt")
                    load_img([nc.sync, nc.scalar], 3, xab, [(0, PHW)])
                    imgs[3] = new_img(3, xab)
                emit_dw_pair(img, 1, stat_from_psum=(n == 3))
                emit_dw_pair(img, 2, stat_from_psum=(n == 3))
                # previous image's pw is split around pair 3: each chunk's
                # drains then hide under ~2.3us of dw matmuls
                if n > 0:
                    emit_pw(imgs[n - 1], [pwps_pool], chunks=(0,))
                emit_dw_pair(img, 3, stat_from_psum=(n == 3))
                if n > 0:
                    emit_pw(imgs[n - 1], [pwps_pool], chunks=(1,))
                emit_chain(img, thr1 if n == 3 else float(DW_THRESH))
            # cover image 3's keep chain with dummy matmuls, then its pw
            # with both PSUM pools for deeper pipelining
            dps = dwps_pool.tile([128, 1024], f32, name="dwps")
            warm_mm(4, dps)
            emit_pw(imgs[3], [pwps_pool, dwps_pool], tail=True)

    nc.compile()
    return nc


def _get_nc():
    if "nc" not in _CACHE:
        _CACHE["nc"] = _build()
    return _CACHE["nc"]


def _fold_weights(inputs):
    """Host-side numpy prep of all the small weight algebra."""
    dw_w = np.asarray(inputs["dw_w"], np.float64).reshape(C_IN, 9)
    dw_b = np.asarray(inputs["dw_b"], np.float64)
    g1 = np.asarray(inputs["bn1_g"], np.float64)
    b1 = np.asarray(inputs["bn1_b"], np.float64)
    m1 = np.asarray(inputs["bn1_m"], np.float64)
    v1 = np.asarray(inputs["bn1_v"], np.float64)
    pw_w = np.asarray(inputs["pw_w"], np.float64)
    pw_b = np.asarray(inputs["pw_b"], np.float64)
    g2 = np.asarray(inputs["bn2_g"], np.float64)
    b2 = np.asarray(inputs["bn2_b"], np.float64)
    m2 = np.asarray(inputs["bn2_m"], np.float64)
    v2 = np.asarray(inputs["bn2_v"], np.float64)

    s1 = g1 / np.sqrt(v1 + BN_EPS)
    bias1 = (s1 * (dw_b - m1) + b1).astype(np.float64)
    thr1 = (DW_THRESH - bias1).astype(np.float64)
    dws = dw_w * s1[:, None]                      # [C_IN, 9], k=(di+1)*3+dj+1

    # col-tile weight layout [128, pass, col-tile j, 64]:
    #   centers (pass 0-2): rows 0-63 diag of tap (di,0), rows 64-127 zero
    #   pairs   (pass 3-5): rows 0-63 diag of (di,-1), rows 64-127 (di,+1)
    wcol = np.zeros((C_IN, 6, 2, 64), np.float32)
    d64 = np.arange(64)
    for pidx, (kind, di) in enumerate(DW_PASSES):
        for j in range(2):
            cs = 64 * j
            if kind == "c":
                k = (di + 1) * 3 + 1
                wcol[0:64, pidx, j][d64, d64] = dws[cs + d64, k]
            else:
                kl = (di + 1) * 3 + 0
                kr = (di + 1) * 3 + 2
                wcol[0:64, pidx, j][d64, d64] = dws[cs + d64, kl]
                wcol[64:128, pidx, j][d64, d64] = dws[cs + d64, kr]

    s2 = g2 / np.sqrt(v2 + BN_EPS)
    bias2 = (s2 * (pw_b - m2) + b2).astype(np.float64)
    lhsTb = (pw_w * s2[:, None]).T.astype(np.float32)   # [C_IN, C_OUT]

    biasv = np.stack([bias1, thr1, bias2[:C_IN], bias2[C_IN:]],
                     axis=1).astype(np.float32)          # [C_IN, 4]

    import ml_dtypes
    return {
        "wcol": np.ascontiguousarray(wcol.astype(ml_dtypes.bfloat16)),
        "biasv": np.ascontiguousarray(biasv),
        "lhsTb": np.ascontiguousarray(lhsTb.astype(ml_dtypes.bfloat16)),
    }


def _make_in_maps(inputs):
    import ml_dtypes
    x = np.asarray(inputs["x"]).astype(ml_dtypes.bfloat16)
    # zero-pad each 56x56 map into a 58x58 frame on host, then build the
    # duplicated-shifted planes in their exact SBUF layout
    xp = np.zeros((B, C_IN, HP, WP), dtype=ml_dtypes.bfloat16)
    xp[:, :, 1:57, 1:57] = x
    xpf = np.zeros((B, C_IN, PXW), dtype=ml_dtypes.bfloat16)
    xpf[:, :, 0:PHW] = xp.reshape(B, C_IN, PHW)
    xq = np.empty((B, 2, C_IN, PHW), dtype=ml_dtypes.bfloat16)
    xq[:, 0, 0:64] = xpf[:, 0:64, 0:PHW]
    xq[:, 0, 64:128] = xpf[:, 0:64, 2:PHW + 2]
    xq[:, 1, 0:64] = xpf[:, 64:128, 0:PHW]
    xq[:, 1, 64:128] = xpf[:, 64:128, 2:PHW + 2]
    folded = _fold_weights(inputs)
    in_maps = []
    for c in range(N_CORES):
        m = {"xq": np.ascontiguousarray(xq[c * BPC:(c + 1) * BPC])}
        m.update(folded)
        in_maps.append(m)
    return in_maps


def kernel(**inputs):
    from concourse.bass_utils import run_bass_kernel_spmd

    nc = _get_nc()
    in_maps = _make_in_maps(inputs)
    res = run_bass_kernel_spmd(nc, in_maps, core_ids=list(range(N_CORES)))
    _CACHE["last_results"] = res
    z = np.concatenate([np.asarray(res.results[c]["z"])
                        for c in range(N_CORES)], axis=0).astype(np.float32)
    # pw map-cut on host: zero any (n, o) map whose max is below PW_THRESH
    mx = z.max(axis=(2, 3))
    z *= (mx >= PW_THRESH).astype(np.float32)[:, :, None, None]
    return z


# revision 25
# speedup vs baseline: 1.0246x; 1.0246x over previous
"""Trainium2 Bass kernel: DepthSeparableConv2d (dw3x3 + BN + ReLU + map-cut,
pw 1x1 + BN + ReLU + map-cut), data-parallel over batch on 8 NeuronCores.

Host side folds all the small weight algebra (BN scales into conv weights,
pw transpose, biases, the dw-cut threshold) in numpy and zero-pads x to
58x58 per channel; the device kernel is a pure streaming pipeline per core
(4 images):

  - depthwise 3x3 conv in SIX TensorEngine passes per 448-px tile instead
    of nine: the three horizontal tap pairs (di,-1)+(di,+1) are fused into
    one pass each using 128x64 COLUMN-TILED matmuls.  Two concurrent
    col-tile matmuls (tile_position (0,0) and (0,64)) contract 128-deep
    streams XA = [x_lo; x_lo shifted +2 elems] and XB = [x_hi; x_hi +2]
    (partition-duplicated planes built by the input DMA from the padded
    HBM tensor), so one pass applies BOTH taps of a pair for all 128
    channels.  The three center taps ride the same mode with zeroed
    hi-row weights.  Measured: the second col-tile matmul executes in the
    ~4ns shadow of the first, so a 2-tap pass costs one matmul slot.
  - zero padding lives in the HBM layout (host-padded), so every matmul
    is a full 8x56 window -- no border clipping, no edge fixups.
  - PSUM tiles are 2-bank [128,1024] pairs; each pair drains in ONE
    Scalar/Vector instruction (relu(psum+bias), bf16), and the dw map-cut
    stat is one VectorE XY-reduce straight from PSUM for the last image.
    keep is folded into the pointwise lhsT halves.
  - pointwise 1x1: 2 chunks of 128 out-channels, each emitted as two
    concurrent 128x64 col-tile matmuls so the PE never switches tiling
    mode (mode switches drain the array); paired drains
    relu(psum+bias2) -> bf16 z, ONE output DMA per chunk; the last
    image's chunks go out as two halves on two queues.
  - pw map-cut is applied ON HOST (exact max >= 0.001 test in numpy);
    output DMA is bf16 (half traffic), host casts to fp32.
  - schedule: pw(n) is emitted BETWEEN pairs 2 and 3 of dw(n+1); image 0's
    input load is split into 3 row-chunks x 2 planes across the sync /
    gpsimd / scalar / vector queues so the first dw pair starts as early
    as possible.
"""

import numpy as np

B, C_IN, C_OUT, H, W = 32, 128, 256, 56, 56
N_CORES = 8
BPC = B // N_CORES          # images per core
HW = H * W                  # 3136
TILE_ROWS = 8               # output rows per 448-px sub-tile
NT = H // TILE_ROWS         # 7 sub-tiles per image
TN = TILE_ROWS * W          # 448 pixels per sub-tile
BN_EPS = 1e-5
DW_THRESH = 4.0
PW_THRESH = 0.001

HP, WP = H + 2, W + 2       # padded image 58x58
PHW = HP * WP               # 3364
PXW = PHW + 2               # host row length (2 zeros so the +2-shifted
                            # plane load stays in bounds)

# pairs of 448-px sub-tiles sharing one 2-bank PSUM tile
PAIRS = [(0, 1), (2, 3), (4, 5), (6,)]

# dw passes: (kind, di) with kind 'c' = center tap (0 or +-1, 0) or
# 'p' = horizontal pair (di,-1)+(di,+1).  Center (0,0) first: its two
# col-tile matmuls carry start=True and cover the full tile region.
DW_PASSES = [("c", 0), ("c", -1), ("c", 1), ("p", -1), ("p", 0), ("p", 1)]

_CACHE = {}


def _build():
    import concourse.bacc as bacc
    import concourse.tile as tile
    import concourse.mybir as mybir

    f32 = mybir.dt.float32
    bf16 = mybir.dt.bfloat16
    Alu = mybir.AluOpType
    Act = mybir.ActivationFunctionType

    nc = bacc.Bacc("TRN2", target_bir_lowering=False, debug=False,
                   enable_asserts=False, num_devices=N_CORES)

    # input planes prebuilt on host in exact SBUF layout: plane 0 =
    # [x_lo; x_lo shifted +2 elems], plane 1 = [x_hi; x_hi +2] -- every
    # device read is a full-width, aligned, contiguous DMA
    xp_d = nc.dram_tensor("xq", [BPC, 2, C_IN, PHW], bf16,
                          kind="ExternalInput").ap()
    wc_d = nc.dram_tensor("wcol", [C_IN, 6, 2, 64], bf16, kind="ExternalInput").ap()
    # bias1 / thr1 / bias2-lo / bias2-hi packed as one [C_IN, 4] tensor
    bv_d = nc.dram_tensor("biasv", [C_IN, 4], f32, kind="ExternalInput").ap()
    lw_d = nc.dram_tensor("lhsTb", [C_IN, C_OUT], bf16, kind="ExternalInput").ap()
    z_d = nc.dram_tensor("z", [BPC, C_OUT, H, W], bf16, kind="ExternalOutput").ap()

    TPOS = [(0, 0), (0, 64)]
    HALVES = [(0, 64), (64, 128)]

    with tile.TileContext(nc) as tc:
        with tc.tile_pool(name="const", bufs=1) as cp, \
             tc.tile_pool(name="xb", bufs=3) as xbp, \
             tc.tile_pool(name="y", bufs=3) as yp, \
             tc.tile_pool(name="z", bufs=4) as zp, \
             tc.tile_pool(name="small", bufs=8) as sp, \
             tc.tile_pool(name="dwps", bufs=2, space="PSUM") as dwps_pool, \
             tc.tile_pool(name="pwps", bufs=4, space="PSUM") as pwps_pool:

            # ---- startup: weights on gpsimd queue; image 0 in 3 row-chunks
            # x 2 planes spread across 4 queues so pair 0 can start early ----
            wcol = cp.tile([128, 6, 2, 64], bf16)
            nc.gpsimd.dma_start(wcol[:], wc_d)
            bv = cp.tile([128, 4], f32)
            nc.gpsimd.dma_start(bv[:], bv_d)
            bias1 = bv[:, 0:1]
            thr1 = bv[:, 1:2]
            bias2 = [bv[:, 2:3], bv[:, 3:4]]
            lhsT_base = cp.tile([128, C_OUT], bf16)
            nc.gpsimd.dma_start(lhsT_base[:], lw_d)

            xb0 = xbp.tile([128, 2, PHW], bf16, name="xbt")

            def load_img(qs, n, xab, chunks):
                """DMA image n's two prebuilt planes; one full-partition
                aligned DMA per (chunk, plane)."""
                k = 0
                for (a, b) in chunks:
                    for pl in range(2):
                        qs[k % len(qs)].dma_start(
                            xab[:, pl, a:b], xp_d[n, pl, :, a:b])
                        k += 1

            # chunk 1 covers pair 0 (rows 0-17), chunk 2 pair 1 (rows to 33):
            # both split across sync+scalar, which start their DGEs first.
            # chunk 3 (rows 34-57, needed ~6us later) rides gpsimd behind the
            # small weight loads.
            load_img([nc.sync, nc.scalar], 0, xb0, [(0, 18 * WP)])
            load_img([nc.sync, nc.scalar], 0, xb0, [(18 * WP, 34 * WP)])
            load_img([nc.gpsimd], 0, xb0, [(34 * WP, PHW)])

            # warm the PE HAM clock while the first DMAs are in flight
            # (col-tiled like everything else: mode switches drain the PE)
            warm = cp.tile([128, 448], bf16)
            nc.vector.memset(warm[:], 0.0)
            wps = pwps_pool.tile([128, 512], f32, name="pwps")

            def warm_mm(n_reps, psum):
                for _ in range(n_reps):
                    for ti, (p0, p1) in enumerate(HALVES):
                        nc.tensor.matmul(psum[p0:p1, 0:448], warm[:, 0:64],
                                         warm[:], start=True, stop=True,
                                         tile_position=TPOS[ti],
                                         skip_group_check=True)

            warm_mm(7, wps)

            state = {"dr": 0}

            def drain_op(dst, src, bias, rot):
                state["dr"] += 1
                eng = rot[state["dr"] % len(rot)]
                if eng is nc.scalar:
                    nc.scalar.activation(dst, src, Act.Relu,
                                         bias=bias, scale=1.0)
                else:
                    eng.tensor_scalar(dst, src, bias, 0.0,
                                      Alu.add, Alu.max)

            def emit_dw_pair(img, pi, stat_from_psum=False):
                """one PSUM pair of the depthwise conv: 6 col-tiled passes
                per 448-px tile, keep-stat reduce, paired drain."""
                xab, yb, partdw = img["xb"], img["yb"], img["partdw"]
                pair = img["pairs"][pi]
                ps = dwps_pool.tile([128, 1024], f32, name="dwps")
                x3 = [xab[:, 0, :].rearrange("c (h w) -> c h w", h=HP),
                      xab[:, 1, :].rearrange("c (h w) -> c h w", h=HP)]
                for k, tt in enumerate(pair):
                    r0 = tt * TILE_ROWS
                    ps3 = ps[:, k * 512:k * 512 + TN].rearrange(
                        "c (h w) -> c h w", h=TILE_ROWS)
                    for pidx, (kind, di) in enumerate(DW_PASSES):
                        rlo = r0 + 1 + di
                        cl = 1 if kind == "c" else 0
                        first = (pidx == 0)
                        last = (pidx == len(DW_PASSES) - 1)
                        for ti, (p0, p1) in enumerate(HALVES):
                            nc.tensor.matmul(
                                ps3[p0:p1],
                                wcol[:, pidx, ti, :],
                                x3[ti][:, rlo:rlo + TILE_ROWS, cl:cl + W],
                                start=first, stop=last,
                                tile_position=TPOS[ti],
                                skip_group_check=True)
                npair = len(pair)
                src = ps[:, 0:npair * 512].rearrange(
                    "c (b x) -> c b x", b=npair)[:, :, 0:TN]
                c0 = pair[0] * TN
                dst = yb[:, c0:c0 + npair * TN].rearrange(
                    "c (b x) -> c b x", b=npair)
                if stat_from_psum:
                    nc.vector.tensor_reduce(partdw[:, pi:pi + 1], src,
                                            axis=mybir.AxisListType.XY,
                                            op=Alu.max)
                drain_op(dst, src, bias1, (nc.scalar,))
                if not stat_from_psum:
                    nc.vector.tensor_reduce(partdw[:, pi:pi + 1],
                                            yb[:, c0:c0 + npair * TN],
                                            axis=mybir.AxisListType.X,
                                            op=Alu.max)
                if pi == 2:
                    nc.vector.tensor_reduce(img["mxa"][:], partdw[:, 0:3],
                                            axis=mybir.AxisListType.X,
                                            op=Alu.max)

            def emit_chain(img, thr=float(DW_THRESH)):
                """keep1 -> masked lhsT halves (all on VectorE)."""
                mx1 = sp.tile([128, 1], f32, name="mx1")
                nc.vector.tensor_max(mx1[:], img["mxa"][:],
                                     img["partdw"][:, 3:4])
                keep1 = sp.tile([128, 1], f32, name="keep1")
                nc.vector.tensor_scalar(keep1[:], mx1[:], thr,
                                        None, Alu.is_ge)
                for m in range(2):
                    lm = sp.tile([128, 128], bf16, name=f"lhsTm{m}")
                    nc.vector.tensor_scalar(
                        lm[:], lhsT_base[:, m * 128:(m + 1) * 128], keep1[:],
                        None, Alu.mult)
                    img["lhsTm"].append(lm)

            def emit_pw(img, tail=False, chunks=(0, 1)):
                """pw out-channel chunks as per-tile col-tiled matmuls with
                single-bank PSUM tiles (4 in flight) so the PE never waits
                on a drain.  Output DMA per chunk; the last image (tail)
                streams drained tile-groups out immediately."""
                n, yb = img["n"], img["yb"]
                for m in chunks:
                    zrow = z_d[n, m * 128:(m + 1) * 128].rearrange(
                        "c h w -> c (h w)")
                    lhsTm = img["lhsTm"][m]
                    zt = zp.tile([128, HW], bf16, name="zt")
                    sent = 0
                    for tt in range(NT):
                        ps = pwps_pool.tile([128, 512], f32, name="pwps")
                        for ti, (p0, p1) in enumerate(HALVES):
                            nc.tensor.matmul(
                                ps[p0:p1, 0:TN], lhsTm[:, p0:p1],
                                yb[:, tt * TN:(tt + 1) * TN],
                                start=True, stop=True,
                                tile_position=TPOS[ti],
                                skip_group_check=True)
                        drain_op(zt[:, tt * TN:(tt + 1) * TN], ps[:, 0:TN],
                                 bias2[m], (nc.scalar, nc.vector))
                        if tail and tt % 2 == 1:
                            # stream each drained 2-tile group immediately,
                            # alternating queues
                            qa, qb = ((nc.sync, nc.scalar) if m == 0
                                      else (nc.scalar, nc.sync))
                            q = qa if (tt // 2) % 2 == 0 else qb
                            q.dma_start(zrow[:, sent * TN:(tt + 1) * TN],
                                        zt[:, sent * TN:(tt + 1) * TN])
                            sent = tt + 1
                    if tail:
                        (nc.sync if m == 0 else nc.scalar).dma_start(
                            zrow[:, sent * TN:], zt[:, sent * TN:])
                    else:
                        # chunk 0 on sync, chunk 1 on scalar: keeps the sync
                        # queue's per-image byte load under the image period
                        (nc.sync if m == 0 else nc.scalar).dma_start(
                            zrow[:], zt[:])

            def new_img(n, xab):
                if n == 0:
                    pairs = [PAIRS[0], PAIRS[1], PAIRS[3], PAIRS[2]]
                elif n == BPC - 1:
                    pairs = PAIRS
                else:
                    pairs = [PAIRS[3]] + PAIRS[0:3]
                return {"n": n, "xb": xab, "pairs": pairs,
                        "yb": yp.tile([128, HW], bf16, name="ybt"),
                        "partdw": sp.tile([128, 4], f32, name="partdw"),
                        "mxa": sp.tile([128, 1], f32, name="mxa"),
                        "lhsTm": []}

            imgs = [None] * BPC
            imgs[0] = new_img(0, xb0)
            # image 1's planes load right behind image 0's chunks (sync +
            # scalar queues are FIFO, so these start as soon as img0 is in)
            xab1 = xbp.tile([128, 2, PHW], bf16, name="xbt")
            load_img([nc.sync, nc.scalar], 1, xab1, [(0, PHW)])
            imgs[1] = new_img(1, xab1)
            for n in range(BPC):
                img = imgs[n]
                emit_dw_pair(img, 0, stat_from_psum=(n == 3))
                # prefetch image n+2 == 2 on the idle gpsimd queue (its tile
                # buffer is fresh, so the DMA carries no semaphore waits --
                # waiting DMAs on the gpsimd queue hang); image 3 reuses a
                # buffer and goes on sync+scalar
                if n == 0:
                    xab = xbp.tile([128, 2, PHW], bf16, name="xbt")
                    load_img([nc.gpsimd], 2, xab, [(0, PHW)])
                    imgs[2] = new_img(2, xab)
                elif n == 1:
                    xab = xbp.tile([128, 2, PHW], bf16, name="xbt")
                    load_img([nc.sync, nc.scalar], 3, xab, [(0, PHW)])
                    imgs[3] = new_img(3, xab)
                emit_dw_pair(img, 1, stat_from_psum=(n == 3))
                emit_dw_pair(img, 2, stat_from_psum=(n == 3))
                # previous image's pw is split around pair 3: each chunk's
                # drains then hide under ~2.3us of dw matmuls
                if n > 0:
                    emit_pw(imgs[n - 1], chunks=(0,))
                emit_dw_pair(img, 3, stat_from_psum=(n == 3))
                if n > 0:
                    emit_pw(imgs[n - 1], chunks=(1,))
                emit_chain(img, thr1 if n == 3 else float(DW_THRESH))
            # cover image 3's keep chain with dummy matmuls, then its pw
            # with both PSUM pools for deeper pipelining
            dps = dwps_pool.tile([128, 1024], f32, name="dwps")
            warm_mm(4, dps)
            emit_pw(imgs[3], tail=True)

    nc.compile()
    return nc


def _get_nc():
    if "nc" not in _CACHE:
        _CACHE["nc"] = _build()
    return _CACHE["nc"]


def _fold_weights(inputs):
    """Host-side numpy prep of all the small weight algebra."""
    dw_w = np.asarray(inputs["dw_w"], np.float64).reshape(C_IN, 9)
    dw_b = np.asarray(inputs["dw_b"], np.float64)
    g1 = np.asarray(inputs["bn1_g"], np.float64)
    b1 = np.asarray(inputs["bn1_b"], np.float64)
    m1 = np.asarray(inputs["bn1_m"], np.float64)
    v1 = np.asarray(inputs["bn1_v"], np.float64)
    pw_w = np.asarray(inputs["pw_w"], np.float64)
    pw_b = np.asarray(inputs["pw_b"], np.float64)
    g2 = np.asarray(inputs["bn2_g"], np.float64)
    b2 = np.asarray(inputs["bn2_b"], np.float64)
    m2 = np.asarray(inputs["bn2_m"], np.float64)
    v2 = np.asarray(inputs["bn2_v"], np.float64)

    s1 = g1 / np.sqrt(v1 + BN_EPS)
    bias1 = (s1 * (dw_b - m1) + b1).astype(np.float64)
    thr1 = (DW_THRESH - bias1).astype(np.float64)
    dws = dw_w * s1[:, None]                      # [C_IN, 9], k=(di+1)*3+dj+1

    # col-tile weight layout [128, pass, col-tile j, 64]:
    #   centers (pass 0-2): rows 0-63 diag of tap (di,0), rows 64-127 zero
    #   pairs   (pass 3-5): rows 0-63 diag of (di,-1), rows 64-127 (di,+1)
    wcol = np.zeros((C_IN, 6, 2, 64), np.float32)
    d64 = np.arange(64)
    for pidx, (kind, di) in enumerate(DW_PASSES):
        for j in range(2):
            cs = 64 * j
            if kind == "c":
                k = (di + 1) * 3 + 1
                wcol[0:64, pidx, j][d64, d64] = dws[cs + d64, k]
            else:
                kl = (di + 1) * 3 + 0
                kr = (di + 1) * 3 + 2
                wcol[0:64, pidx, j][d64, d64] = dws[cs + d64, kl]
                wcol[64:128, pidx, j][d64, d64] = dws[cs + d64, kr]

    s2 = g2 / np.sqrt(v2 + BN_EPS)
    bias2 = (s2 * (pw_b - m2) + b2).astype(np.float64)
    lhsTb = (pw_w * s2[:, None]).T.astype(np.float32)   # [C_IN, C_OUT]

    biasv = np.stack([bias1, thr1, bias2[:C_IN], bias2[C_IN:]],
                     axis=1).astype(np.float32)          # [C_IN, 4]

    import ml_dtypes
    return {
        "wcol": np.ascontiguousarray(wcol.astype(ml_dtypes.bfloat16)),
        "biasv": np.ascontiguousarray(biasv),
        "lhsTb": np.ascontiguousarray(lhsTb.astype(ml_dtypes.bfloat16)),
    }


def _make_in_maps(inputs):
    import ml_dtypes
    x = np.asarray(inputs["x"]).astype(ml_dtypes.bfloat16)
    # zero-pad each 56x56 map into a 58x58 frame on host, then build the
    # duplicated-shifted planes in their exact SBUF layout
    xp = np.zeros((B, C_IN, HP, WP), dtype=ml_dtypes.bfloat16)
    xp[:, :, 1:57, 1:57] = x
    xpf = np.zeros((B, C_IN, PXW), dtype=ml_dtypes.bfloat16)
    xpf[:, :, 0:PHW] = xp.reshape(B, C_IN, PHW)
    xq = np.empty((B, 2, C_IN, PHW), dtype=ml_dtypes.bfloat16)
    xq[:, 0, 0:64] = xpf[:, 0:64, 0:PHW]
    xq[:, 0, 64:128] = xpf[:, 0:64, 2:PHW + 2]
    xq[:, 1, 0:64] = xpf[:, 64:128, 0:PHW]
    xq[:, 1, 64:128] = xpf[:, 64:128, 2:PHW + 2]
    folded = _fold_weights(inputs)
    in_maps = []
    for c in range(N_CORES):
        m = {"xq": np.ascontiguousarray(xq[c * BPC:(c + 1) * BPC])}
        m.update(folded)
        in_maps.append(m)
    return in_maps


def kernel(**inputs):
    from concourse.bass_utils import run_bass_kernel_spmd

    nc = _get_nc()
    in_maps = _make_in_maps(inputs)
    res = run_bass_kernel_spmd(nc, in_maps, core_ids=list(range(N_CORES)))
    _CACHE["last_results"] = res
    z = np.concatenate([np.asarray(res.results[c]["z"])
                        for c in range(N_CORES)], axis=0).astype(np.float32)
    # pw map-cut on host: zero any (n, o) map whose max is below PW_THRESH
    mx = z.max(axis=(2, 3))
    z *= (mx >= PW_THRESH).astype(np.float32)[:, :, None, None]
    return z


# revision 28
# speedup vs baseline: 1.0863x; 1.0602x over previous
"""Trainium2 Bass kernel: DepthSeparableConv2d (dw3x3 + BN + ReLU + map-cut,
pw 1x1 + BN + ReLU + map-cut), data-parallel over batch on 8 NeuronCores.

Host side folds all the small weight algebra (BN scales into conv weights,
pw transpose, biases, the dw-cut threshold) in numpy and zero-pads x to
58x58 per channel; the device kernel is a pure streaming pipeline per core
(4 images):

  - depthwise 3x3 conv in SIX TensorEngine passes per 448-px tile instead
    of nine: the three horizontal tap pairs (di,-1)+(di,+1) are fused into
    one pass each using 128x64 COLUMN-TILED matmuls.  Two concurrent
    col-tile matmuls (tile_position (0,0) and (0,64)) contract 128-deep
    streams XA = [x_lo; x_lo shifted +2 elems] and XB = [x_hi; x_hi +2]
    (partition-duplicated planes built by the input DMA from the padded
    HBM tensor), so one pass applies BOTH taps of a pair for all 128
    channels.  The three center taps ride the same mode with zeroed
    hi-row weights.  Measured: the second col-tile matmul executes in the
    ~4ns shadow of the first, so a 2-tap pass costs one matmul slot.
  - zero padding lives in the HBM layout (host-padded), so every matmul
    is a full 8x56 window -- no border clipping, no edge fixups.
  - PSUM tiles are 2-bank [128,1024] pairs; each pair drains in ONE
    Scalar/Vector instruction (relu(psum+bias), bf16), and the dw map-cut
    stat is one VectorE XY-reduce straight from PSUM for the last image.
    keep is folded into the pointwise lhsT halves.
  - pointwise 1x1: 2 chunks of 128 out-channels, each emitted as two
    concurrent 128x64 col-tile matmuls so the PE never switches tiling
    mode (mode switches drain the array); paired drains
    relu(psum+bias2) -> bf16 z, ONE output DMA per chunk; the last
    image's chunks go out as two halves on two queues.
  - pw map-cut is applied ON HOST (exact max >= 0.001 test in numpy);
    output DMA is bf16 (half traffic), host casts to fp32.
  - schedule: pw(n) is emitted BETWEEN pairs 2 and 3 of dw(n+1); image 0's
    input load is split into 3 row-chunks x 2 planes across the sync /
    gpsimd / scalar / vector queues so the first dw pair starts as early
    as possible.
"""

import numpy as np

B, C_IN, C_OUT, H, W = 32, 128, 256, 56, 56
N_CORES = 8
BPC = B // N_CORES          # images per core
HW = H * W                  # 3136
TILE_ROWS = 8               # output rows per 448-px sub-tile
NT = H // TILE_ROWS         # 7 sub-tiles per image
TN = TILE_ROWS * W          # 448 pixels per sub-tile
BN_EPS = 1e-5
DW_THRESH = 4.0
PW_THRESH = 0.001

HP, WP = H + 2, W + 2       # padded image 58x58
PHW = HP * WP               # 3364
PXW = PHW + 2               # host row length (2 zeros so the +2-shifted
                            # plane load stays in bounds)

# pairs of 448-px sub-tiles sharing one 2-bank PSUM tile
PAIRS = [(0, 1), (2, 3), (4, 5), (6,)]

# dw passes: (kind, di) with kind 'c' = center tap (0 or +-1, 0) or
# 'p' = horizontal pair (di,-1)+(di,+1).  Center (0,0) first: its two
# col-tile matmuls carry start=True and cover the full tile region.
DW_PASSES = [("c", 0), ("c", -1), ("c", 1), ("p", -1), ("p", 0), ("p", 1)]

_CACHE = {}


def _build():
    import concourse.bacc as bacc
    import concourse.tile as tile
    import concourse.mybir as mybir

    f32 = mybir.dt.float32
    bf16 = mybir.dt.bfloat16
    Alu = mybir.AluOpType
    Act = mybir.ActivationFunctionType

    nc = bacc.Bacc("TRN2", target_bir_lowering=False, debug=False,
                   enable_asserts=False, num_devices=N_CORES)

    # input planes prebuilt on host in exact SBUF layout: plane 0 =
    # [x_lo; x_lo shifted +2 elems], plane 1 = [x_hi; x_hi +2] -- every
    # device read is a full-width, aligned, contiguous DMA
    xp_d = nc.dram_tensor("xq", [BPC, 2, C_IN, PHW], bf16,
                          kind="ExternalInput").ap()
    wc_d = nc.dram_tensor("wcol", [C_IN, 6, 2, 64], bf16, kind="ExternalInput").ap()
    # bias1 / thr1 / bias2-lo / bias2-hi packed as one [C_IN, 4] tensor
    bv_d = nc.dram_tensor("biasv", [C_IN, 4], f32, kind="ExternalInput").ap()
    lw_d = nc.dram_tensor("lhsTb", [C_IN, C_OUT], bf16, kind="ExternalInput").ap()
    z_d = nc.dram_tensor("z", [BPC, C_OUT, H, W], bf16, kind="ExternalOutput").ap()

    TPOS = [(0, 0), (0, 64)]
    HALVES = [(0, 64), (64, 128)]

    with tile.TileContext(nc) as tc:
        with tc.tile_pool(name="const", bufs=1) as cp, \
             tc.tile_pool(name="xb", bufs=3) as xbp, \
             tc.tile_pool(name="y", bufs=3) as yp, \
             tc.tile_pool(name="z", bufs=4) as zp, \
             tc.tile_pool(name="small", bufs=8) as sp, \
             tc.tile_pool(name="dwps", bufs=2, space="PSUM") as dwps_pool, \
             tc.tile_pool(name="pwps", bufs=4, space="PSUM") as pwps_pool:

            # ---- startup: weights on gpsimd queue; image 0 in 3 row-chunks
            # x 2 planes spread across 4 queues so pair 0 can start early ----
            wcol = cp.tile([128, 6, 2, 64], bf16)
            nc.gpsimd.dma_start(wcol[:], wc_d)
            bv = cp.tile([128, 4], f32)
            nc.gpsimd.dma_start(bv[:], bv_d)
            bias1 = bv[:, 0:1]
            thr1 = bv[:, 1:2]
            bias2 = [bv[:, 2:3], bv[:, 3:4]]
            lhsT_base = cp.tile([128, C_OUT], bf16)
            nc.gpsimd.dma_start(lhsT_base[:], lw_d)

            xb0 = xbp.tile([128, 2, PHW], bf16, name="xbt")

            def load_img(qs, n, xab, chunks):
                """DMA image n's two prebuilt planes; one full-partition
                aligned DMA per (chunk, plane)."""
                k = 0
                for (a, b) in chunks:
                    for pl in range(2):
                        qs[k % len(qs)].dma_start(
                            xab[:, pl, a:b], xp_d[n, pl, :, a:b])
                        k += 1

            # image 0 entirely on sync (plane 0) + scalar (plane 1), in four
            # row-chunks so the first dw tile (rows 0-9) can start the moment
            # ~0.15 MB has landed
            load_img([nc.sync, nc.scalar], 0, xb0,
                     [(0, 10 * WP), (10 * WP, 18 * WP),
                      (18 * WP, 34 * WP), (34 * WP, PHW)])

            # warm the PE HAM clock while the first DMAs are in flight
            # (col-tiled like everything else: mode switches drain the PE)
            warm = cp.tile([128, 448], bf16)
            nc.vector.memset(warm[:], 0.0)
            wps = pwps_pool.tile([128, 512], f32, name="pwps")

            def warm_mm(n_reps, psum):
                for _ in range(n_reps):
                    for ti, (p0, p1) in enumerate(HALVES):
                        nc.tensor.matmul(psum[p0:p1, 0:448], warm[:, 0:64],
                                         warm[:], start=True, stop=True,
                                         tile_position=TPOS[ti],
                                         skip_group_check=True)

            warm_mm(5, wps)

            state = {"dr": 0}

            def drain_op(dst, src, bias, rot):
                state["dr"] += 1
                eng = rot[state["dr"] % len(rot)]
                if eng is nc.scalar:
                    nc.scalar.activation(dst, src, Act.Relu,
                                         bias=bias, scale=1.0)
                else:
                    eng.tensor_scalar(dst, src, bias, 0.0,
                                      Alu.add, Alu.max)

            def emit_dw_pair(img, pi, stat_from_psum=False):
                """one PSUM pair of the depthwise conv: 6 col-tiled passes
                per 448-px tile, keep-stat reduce, paired drain."""
                xab, yb, partdw = img["xb"], img["yb"], img["partdw"]
                pair = img["pairs"][pi]
                ps = dwps_pool.tile([128, 1024], f32, name="dwps")
                x3 = [xab[:, 0, :].rearrange("c (h w) -> c h w", h=HP),
                      xab[:, 1, :].rearrange("c (h w) -> c h w", h=HP)]
                for k, tt in enumerate(pair):
                    r0 = tt * TILE_ROWS
                    ps3 = ps[:, k * 512:k * 512 + TN].rearrange(
                        "c (h w) -> c h w", h=TILE_ROWS)
                    for pidx, (kind, di) in enumerate(DW_PASSES):
                        rlo = r0 + 1 + di
                        cl = 1 if kind == "c" else 0
                        first = (pidx == 0)
                        last = (pidx == len(DW_PASSES) - 1)
                        for ti, (p0, p1) in enumerate(HALVES):
                            nc.tensor.matmul(
                                ps3[p0:p1],
                                wcol[:, pidx, ti, :],
                                x3[ti][:, rlo:rlo + TILE_ROWS, cl:cl + W],
                                start=first, stop=last,
                                tile_position=TPOS[ti],
                                skip_group_check=True)
                npair = len(pair)
                src = ps[:, 0:npair * 512].rearrange(
                    "c (b x) -> c b x", b=npair)[:, :, 0:TN]
                c0 = pair[0] * TN
                dst = yb[:, c0:c0 + npair * TN].rearrange(
                    "c (b x) -> c b x", b=npair)
                if stat_from_psum:
                    nc.vector.tensor_reduce(partdw[:, pi:pi + 1], src,
                                            axis=mybir.AxisListType.XY,
                                            op=Alu.max)
                drain_op(dst, src, bias1, (nc.scalar,))
                if not stat_from_psum:
                    nc.vector.tensor_reduce(partdw[:, pi:pi + 1],
                                            yb[:, c0:c0 + npair * TN],
                                            axis=mybir.AxisListType.X,
                                            op=Alu.max)
                if pi == 2:
                    nc.vector.tensor_reduce(img["mxa"][:], partdw[:, 0:3],
                                            axis=mybir.AxisListType.X,
                                            op=Alu.max)

            def emit_chain(img, thr=float(DW_THRESH)):
                """keep1 -> masked lhsT halves (all on VectorE)."""
                mx1 = sp.tile([128, 1], f32, name="mx1")
                nc.vector.tensor_max(mx1[:], img["mxa"][:],
                                     img["partdw"][:, 3:4])
                keep1 = sp.tile([128, 1], f32, name="keep1")
                nc.vector.tensor_scalar(keep1[:], mx1[:], thr,
                                        None, Alu.is_ge)
                for m in range(2):
                    lm = sp.tile([128, 128], bf16, name=f"lhsTm{m}")
                    nc.vector.tensor_scalar(
                        lm[:], lhsT_base[:, m * 128:(m + 1) * 128], keep1[:],
                        None, Alu.mult)
                    img["lhsTm"].append(lm)

            def emit_pw(img, tail=False, chunks=(0, 1)):
                """pw out-channel chunks as per-tile col-tiled matmuls with
                single-bank PSUM tiles (4 in flight) so the PE never waits
                on a drain.  Output DMA per chunk; the last image (tail)
                streams drained tile-groups out immediately."""
                n, yb = img["n"], img["yb"]
                for m in chunks:
                    zrow = z_d[n, m * 128:(m + 1) * 128].rearrange(
                        "c h w -> c (h w)")
                    lhsTm = img["lhsTm"][m]
                    zt = zp.tile([128, HW], bf16, name="zt")
                    sent = 0
                    for tt in range(NT):
                        ps = pwps_pool.tile([128, 512], f32, name="pwps")
                        for ti, (p0, p1) in enumerate(HALVES):
                            nc.tensor.matmul(
                                ps[p0:p1, 0:TN], lhsTm[:, p0:p1],
                                yb[:, tt * TN:(tt + 1) * TN],
                                start=True, stop=True,
                                tile_position=TPOS[ti],
                                skip_group_check=True)
                        drain_op(zt[:, tt * TN:(tt + 1) * TN], ps[:, 0:TN],
                                 bias2[m], (nc.scalar, nc.vector))
                        if tail and tt % 2 == 1:
                            # stream each drained 2-tile group immediately,
                            # alternating queues
                            qa, qb = ((nc.sync, nc.scalar) if m == 0
                                      else (nc.scalar, nc.sync))
                            q = qa if (tt // 2) % 2 == 0 else qb
                            q.dma_start(zrow[:, sent * TN:(tt + 1) * TN],
                                        zt[:, sent * TN:(tt + 1) * TN])
                            sent = tt + 1
                    if tail:
                        (nc.sync if m == 0 else nc.scalar).dma_start(
                            zrow[:, sent * TN:], zt[:, sent * TN:])
                    else:
                        # chunk 0 on sync, chunk 1 on scalar: keeps the sync
                        # queue's per-image byte load under the image period
                        (nc.sync if m == 0 else nc.scalar).dma_start(
                            zrow[:], zt[:])

            def new_img(n, xab):
                if n == 0:
                    pairs = [PAIRS[0], PAIRS[1], PAIRS[3], PAIRS[2]]
                elif n == BPC - 1:
                    pairs = PAIRS
                else:
                    pairs = [PAIRS[3]] + PAIRS[0:3]
                return {"n": n, "xb": xab, "pairs": pairs,
                        "yb": yp.tile([128, HW], bf16, name="ybt"),
                        "partdw": sp.tile([128, 4], f32, name="partdw"),
                        "mxa": sp.tile([128, 1], f32, name="mxa"),
                        "lhsTm": []}

            imgs = [None] * BPC
            imgs[0] = new_img(0, xb0)
            # image 1: first 18 rows ride the gpsimd queue right behind the
            # small weight loads (wait-free: fresh buffer), the rest queues
            # behind image 0 on sync+scalar
            xab1 = xbp.tile([128, 2, PHW], bf16, name="xbt")
            load_img([nc.gpsimd], 1, xab1, [(0, 18 * WP)])
            load_img([nc.sync, nc.scalar], 1, xab1, [(18 * WP, PHW)])
            imgs[1] = new_img(1, xab1)
            for n in range(BPC):
                img = imgs[n]
                emit_dw_pair(img, 0, stat_from_psum=(n == 3))
                # prefetch image n+2 == 2 on the idle gpsimd queue (its tile
                # buffer is fresh, so the DMA carries no semaphore waits --
                # waiting DMAs on the gpsimd queue hang); image 3 reuses a
                # buffer and goes on sync+scalar
                if n == 0:
                    xab = xbp.tile([128, 2, PHW], bf16, name="xbt")
                    load_img([nc.gpsimd], 2, xab, [(0, PHW)])
                    imgs[2] = new_img(2, xab)
                elif n == 1:
                    xab = xbp.tile([128, 2, PHW], bf16, name="xbt")
                    load_img([nc.sync, nc.scalar], 3, xab, [(0, PHW)])
                    imgs[3] = new_img(3, xab)
                emit_dw_pair(img, 1, stat_from_psum=(n == 3))
                emit_dw_pair(img, 2, stat_from_psum=(n == 3))
                # previous image's pw is split around pair 3: each chunk's
                # drains then hide under ~2.3us of dw matmuls
                if n > 0:
                    emit_pw(imgs[n - 1], chunks=(0,))
                emit_dw_pair(img, 3, stat_from_psum=(n == 3))
                if n > 0:
                    emit_pw(imgs[n - 1], chunks=(1,))
                emit_chain(img, thr1 if n == 3 else float(DW_THRESH))
            # cover image 3's keep chain with dummy matmuls, then its pw
            # with both PSUM pools for deeper pipelining
            dps = dwps_pool.tile([128, 1024], f32, name="dwps")
            warm_mm(4, dps)
            emit_pw(imgs[3], tail=True)

    nc.compile()
    return nc


def _get_nc():
    if "nc" not in _CACHE:
        _CACHE["nc"] = _build()
    return _CACHE["nc"]


def _fold_weights(inputs):
    """Host-side numpy prep of all the small weight algebra."""
    dw_w = np.asarray(inputs["dw_w"], np.float64).reshape(C_IN, 9)
    dw_b = np.asarray(inputs["dw_b"], np.float64)
    g1 = np.asarray(inputs["bn1_g"], np.float64)
    b1 = np.asarray(inputs["bn1_b"], np.float64)
    m1 = np.asarray(inputs["bn1_m"], np.float64)
    v1 = np.asarray(inputs["bn1_v"], np.float64)
    pw_w = np.asarray(inputs["pw_w"], np.float64)
    pw_b = np.asarray(inputs["pw_b"], np.float64)
    g2 = np.asarray(inputs["bn2_g"], np.float64)
    b2 = np.asarray(inputs["bn2_b"], np.float64)
    m2 = np.asarray(inputs["bn2_m"], np.float64)
    v2 = np.asarray(inputs["bn2_v"], np.float64)

    s1 = g1 / np.sqrt(v1 + BN_EPS)
    bias1 = (s1 * (dw_b - m1) + b1).astype(np.float64)
    thr1 = (DW_THRESH - bias1).astype(np.float64)
    dws = dw_w * s1[:, None]                      # [C_IN, 9], k=(di+1)*3+dj+1

    # col-tile weight layout [128, pass, col-tile j, 64]:
    #   centers (pass 0-2): rows 0-63 diag of tap (di,0), rows 64-127 zero
    #   pairs   (pass 3-5): rows 0-63 diag of (di,-1), rows 64-127 (di,+1)
    wcol = np.zeros((C_IN, 6, 2, 64), np.float32)
    d64 = np.arange(64)
    for pidx, (kind, di) in enumerate(DW_PASSES):
        for j in range(2):
            cs = 64 * j
            if kind == "c":
                k = (di + 1) * 3 + 1
                wcol[0:64, pidx, j][d64, d64] = dws[cs + d64, k]
            else:
                kl = (di + 1) * 3 + 0
                kr = (di + 1) * 3 + 2
                wcol[0:64, pidx, j][d64, d64] = dws[cs + d64, kl]
                wcol[64:128, pidx, j][d64, d64] = dws[cs + d64, kr]

    s2 = g2 / np.sqrt(v2 + BN_EPS)
    bias2 = (s2 * (pw_b - m2) + b2).astype(np.float64)
    lhsTb = (pw_w * s2[:, None]).T.astype(np.float32)   # [C_IN, C_OUT]

    biasv = np.stack([bias1, thr1, bias2[:C_IN], bias2[C_IN:]],
                     axis=1).astype(np.float32)          # [C_IN, 4]

    import ml_dtypes
    return {
        "wcol": np.ascontiguousarray(wcol.astype(ml_dtypes.bfloat16)),
        "biasv": np.ascontiguousarray(biasv),
        "lhsTb": np.ascontiguousarray(lhsTb.astype(ml_dtypes.bfloat16)),
    }


def _make_in_maps(inputs):
    import ml_dtypes
    x = np.asarray(inputs["x"]).astype(ml_dtypes.bfloat16)
    # zero-pad each 56x56 map into a 58x58 frame on host, then build the
    # duplicated-shifted planes in their exact SBUF layout
    xp = np.zeros((B, C_IN, HP, WP), dtype=ml_dtypes.bfloat16)
    xp[:, :, 1:57, 1:57] = x
    xpf = np.zeros((B, C_IN, PXW), dtype=ml_dtypes.bfloat16)
    xpf[:, :, 0:PHW] = xp.reshape(B, C_IN, PHW)
    xq = np.empty((B, 2, C_IN, PHW), dtype=ml_dtypes.bfloat16)
    xq[:, 0, 0:64] = xpf[:, 0:64, 0:PHW]
    xq[:, 0, 64:128] = xpf[:, 0:64, 2:PHW + 2]
    xq[:, 1, 0:64] = xpf[:, 64:128, 0:PHW]
    xq[:, 1, 64:128] = xpf[:, 64:128, 2:PHW + 2]
    folded = _fold_weights(inputs)
    in_maps = []
    for c in range(N_CORES):
        m = {"xq": np.ascontiguousarray(xq[c * BPC:(c + 1) * BPC])}
        m.update(folded)
        in_maps.append(m)
    return in_maps


def kernel(**inputs):
    from concourse.bass_utils import run_bass_kernel_spmd

    nc = _get_nc()
    in_maps = _make_in_maps(inputs)
    res = run_bass_kernel_spmd(nc, in_maps, core_ids=list(range(N_CORES)))
    _CACHE["last_results"] = res
    z = np.concatenate([np.asarray(res.results[c]["z"])
                        for c in range(N_CORES)], axis=0).astype(np.float32)
    # pw map-cut on host: zero any (n, o) map whose max is below PW_THRESH
    mx = z.max(axis=(2, 3))
    z *= (mx >= PW_THRESH).astype(np.float32)[:, :, None, None]
    return z
